# revision 19
# baseline (speedup 1.0000x reference)
"""Trainium2 Bass kernel for nn_Attention_32246614458829.

Single-query attention with per-head QK LayerNorm, B=1024, K=512, D=256,
H=8 heads of 32 dims. Pure data parallel over 8 NeuronCores (128 batches
per core); weights replicated.

Math reformulation (validated against the reference in numpy):
  Scores need k = LN_head(x_k @ Wk.T).  Under the softmax, per-(b,h)
  constants drop, so with
      qhat = LN(q)*scale,  qw = qhat*kn_w,  s_qw = sum(qw),
      U'[b,h,:] = sum_d qw[b,h,d]*Wk[32h+d,:] - s_qw[b,h]*M[h,:],
      M[h,:]    = mean_d Wk[32h+d,:],
  the pre-softmax score is
      rstd * raw'   with  raw' = U'.x,
      var = meansq = mean_d (x @ Wk_c.T)^2   (Wk_c head-mean-centered),
      rstd = exp(-0.5*ln(var+eps)).
  The V projection is deferred until after the attention contraction:
      ctx[b,h,:] = sum_i attn[b,h,i]*x_v[b,i,:]   (256-dim)
      y[b,32h+d] = ctx[b,h,:] . Wv[32h+d,:]
      out = LN(y)*n_w @ Wp.T + n_b @ Wp.T  (n_w folded into Wp on host)

v2 data path: x_k / x_v are cast to bf16 AND laid out on the host in the
exact feature-major SBUF tile layouts the matmuls consume.  This halves
HBM traffic (f32 -> bf16) and removes every on-chip DMA transpose; each
octet of 8 batches is two plain 2 MiB HWDGE streams with 16 KiB
contiguous per-partition lines.  Matmul issue order is wave-packed
across the 4 PE column groups (scores and ctx), squares are split
scalar/DVE, and the softmax tail of quad q-1 is emitted at the head of
iteration q so the PE never waits on the activation chain.
"""

import os
import sys
import math

import numpy as np
import ml_dtypes

sys.path.insert(0, "/opt/trn_rl_repo")

DIM, H, HD = 256, 8, 32
EPS = 1e-5
B, K = 1024, 512
NCORES = 8
BL = B // NCORES          # 128 batches per core
QUADS = BL // 4           # 32 quads of 4 batches
GROUPS = BL // 16         # 8 groups of 16 batches (for the hv stage)
OCTS = BL // 8            # 16 octets of 8 batches

_CACHE = {}


def _build(nc_mod, tile_mod, mybir):
    bass = nc_mod
    dt = mybir.dt
    nc = __import__("concourse.bacc", fromlist=["Bacc"]).Bacc(
        "TRN2",
        target_bir_lowering=False,
        debug=False,
        enable_asserts=False,
        num_devices=NCORES,
    )

    # ---- DRAM parameters (per-core shards / replicated consts) ----
    # host-pretransposed activations (bf16), packed per octet so each octet
    # is ONE contiguous 4 MiB DMA (32 KiB per partition line):
    #   xall[o, p, 0:8192]     = xkt[o, dp, b, kp, c, p']
    #                            (= x_k[8o+b, 4p'+kp, 128c+dp], p = dp)
    #   xall[o, p, 8192:16384] = xvt[o, p', b, kp, d]
    #                            (= x_v[8o+b, 4p'+kp, d], p = p')
    xall_d = nc.dram_tensor("xall", [OCTS, 128, 16384], dt.bfloat16,
                            kind="ExternalInput").ap()
    up_d = nc.dram_tensor("up", [128, 2, BL, 32], dt.bfloat16, kind="ExternalInput").ap()
    wkt_d = nc.dram_tensor("wkt", [128, 2, 2, 128], dt.bfloat16, kind="ExternalInput").ap()
    on_d = nc.dram_tensor("on", [128, 2, 32], dt.bfloat16, kind="ExternalInput").ap()
    wvt_d = nc.dram_tensor("wvt", [128, 2, 8, 32], dt.bfloat16, kind="ExternalInput").ap()
    wpt_d = nc.dram_tensor("wpt", [128, 2, 2, 128], dt.bfloat16, kind="ExternalInput").ap()
    bias_d = nc.dram_tensor("bias", [128, 2], dt.float32, kind="ExternalInput").ap()
    ident_d = nc.dram_tensor("ident", [128, 128], dt.float32, kind="ExternalInput").ap()
    out_d = nc.dram_tensor("out", [BL, DIM], dt.float32, kind="ExternalOutput").ap()

    with tile_mod.TileContext(nc) as tc:
        with tc.tile_pool(name="singles", bufs=1) as singles, \
             tc.tile_pool(name="octk", bufs=3) as octk, \
             tc.tile_pool(name="octv", bufs=6) as octv, \
             tc.tile_pool(name="upio", bufs=4) as upio, \
             tc.tile_pool(name="quadmid", bufs=3) as quadmid, \
             tc.tile_pool(name="grp", bufs=2) as grp, \
             tc.tile_pool(name="psA", bufs=2, space="PSUM") as psA, \
             tc.tile_pool(name="psK", bufs=3, space="PSUM") as psK, \
             tc.tile_pool(name="psR", bufs=2, space="PSUM") as psR, \
             tc.tile_pool(name="psM", bufs=1, space="PSUM") as psM:

            # Pin the activation table set to natural_log_exp_and_others
            # (global set id 6): it contains square, ln AND exp, so the
            # whole main loop runs with zero ACT_TABLE_LOAD swaps.
            nc.scalar.add_instruction(mybir.InstLoadActFuncSet(
                name=nc.scalar.bass.get_next_instruction_name(),
                ins=[], outs=[], act_func_set_id=6,
            ))

            # ---- load constants once ----
            wkt_sb = singles.tile([128, 2, 2, 128], dt.bfloat16)
            nc.sync.dma_start(out=wkt_sb, in_=wkt_d)
            on_sb = singles.tile([128, 2, 32], dt.bfloat16)
            nc.sync.dma_start(out=on_sb, in_=on_d)
            wvt_sb = singles.tile([128, 2, 8, 32], dt.bfloat16)
            nc.sync.dma_start(out=wvt_sb, in_=wvt_d)
            wpt_sb = singles.tile([128, 2, 2, 128], dt.bfloat16)
            nc.sync.dma_start(out=wpt_sb, in_=wpt_d)
            bias_sb = singles.tile([128, 2], dt.float32)
            nc.sync.dma_start(out=bias_sb, in_=bias_d)
            ident_sb = singles.tile([128, 128], dt.float32)
            nc.sync.dma_start(out=ident_sb, in_=ident_d)

            # y.T accumulator for the whole core: [feat%128, chunk, group, 16]
            yt_sb = singles.tile([128, 2, GROUPS, 16], dt.bfloat16)
            eps_sb = singles.tile([128, 1], dt.float32)
            nc.vector.memset(eps_sb, EPS)

            oct_tiles = {}
            state = {}

            def issue_load(o):
                """issue octet o's loads 2 octets ahead of first use: xkt
                (short-lived, dies after the raw waves) on the scalar HWDGE
                ring, xvt (must live to the ctx stage) on the sync ring, the
                per-octet slice of the score weights on the scalar ring.
                SWDGE is avoided entirely: the Tile scheduler serializes
                every xbar DMA transpose behind all outstanding SWDGE DMAs."""
                if o >= OCTS:
                    return
                xkt8 = octk.tile([128, 8, 4, 2, 128], dt.bfloat16,
                                 tag="xkt", name="xkt8")
                nc.scalar.dma_start(
                    out=xkt8,
                    in_=xall_d[o][:, 0:8192].rearrange(
                        "p (b kp c q) -> p b kp c q", b=8, kp=4, c=2, q=128))
                xvt8 = octv.tile([128, 8, 4, 256], dt.bfloat16,
                                 tag="xvt", name="xvt8")
                nc.sync.dma_start(
                    out=xvt8,
                    in_=xall_d[o][:, 8192:16384].rearrange(
                        "p (b kp d) -> p b kp d", b=8, kp=4, d=256))
                up8 = upio.tile([128, 2, 8, 32], dt.bfloat16,
                                tag="up8", name="up8")
                nc.scalar.dma_start(out=up8, in_=up_d[:, :, 8 * o:8 * o + 8])
                oct_tiles[o] = (xkt8, xvt8, up8)

            # ---- per-quad pipeline stages (quad q) ----
            # S0 (iter q):   kproj + squares (DVE cast/gp mul + scalar sq)
            # S1 (iter q+1): raw score waves [PE]
            # S2 (iter q+2): msq waves [PE], ln/rstd [scalar], logits [DVE]
            # S3 (iter q+3): exp [scalar], attn.T [sync ring], recip [DVE]
            # S4 (iter q+4): ctx [PE head], ctx scale [DVE], ctx.T [scalar]
            # Every engine input is >=1 full iteration old, so the in-order
            # engine queues never block mid-iteration.

            def emit_kproj(quad):
                o, half = divmod(quad, 2)
                xkt, xvt, up8 = oct_tiles[o]
                boff = half * 4
                ksqt = quadmid.tile([128, 2, 4, 4, 128], dt.bfloat16, tag="ksqt")
                for b in range(4):
                    for m in range(2):
                        kt_ps = psK.tile([128, 512], dt.float32, tag="kt")
                        for c in range(2):
                            nc.tensor.matmul(
                                out=kt_ps,
                                lhsT=wkt_sb[:, c, m],
                                rhs=xkt[:, boff + b, :, c, :],
                                start=(c == 0), stop=(c == 1),
                            )
                        idx = b * 2 + m
                        if idx < 5:
                            ktc = quadmid.tile([128, 512], dt.bfloat16,
                                               tag="ktc")
                            nc.vector.tensor_copy(ktc, kt_ps)
                            if idx < 4:
                                nc.gpsimd.tensor_mul(ksqt[:, m, b], ktc, ktc)
                            else:
                                nc.vector.tensor_mul(ksqt[:, m, b], ktc, ktc)
                        else:
                            nc.scalar.activation(
                                out=ksqt[:, m, b], in_=kt_ps,
                                func=mybir.ActivationFunctionType.Square,
                            )
                return {"ksqt": ksqt, "xkt": xkt, "xvt": xvt, "up8": up8,
                        "boff": boff, "quad": quad}

            def emit_raw(st):
                raw_ps = psR.tile([128, 512], dt.float32, tag="raw")
                for c in range(2):
                    for b in range(4):
                        nc.tensor.matmul(
                            out=raw_ps[32 * b:32 * b + 32],
                            lhsT=st["up8"][:, c, st["boff"] + b],
                            rhs=st["xkt"][:, st["boff"] + b, :, c, :],
                            start=(c == 0), stop=(c == 1),
                            tile_position=(0, 32 * b),
                        )
                st["raw"] = raw_ps

            def emit_msq_ln(st):
                msq_ps = psM.tile([128, 512], dt.float32, tag="msq")
                for c in range(2):
                    for b in range(4):
                        nc.tensor.matmul(
                            out=msq_ps[32 * b:32 * b + 32],
                            lhsT=on_sb[:, c],
                            rhs=st["ksqt"][:, c, b],
                            start=(c == 0), stop=(c == 1),
                            tile_position=(0, 32 * b),
                        )
                lnv = quadmid.tile([128, 512], dt.float32, tag="lnv")
                nc.scalar.activation(
                    out=lnv, in_=msq_ps,
                    func=mybir.ActivationFunctionType.Ln, bias=eps_sb,
                )
                nc.scalar.activation(
                    out=lnv, in_=lnv,
                    func=mybir.ActivationFunctionType.Exp, scale=-0.5,
                )
                st["lnv"] = lnv

            def emit_logits(st):
                nc.vector.tensor_mul(st["lnv"], st["raw"], st["lnv"])

            def emit_tail(st):
                """exp (no max subtraction: |logit| <= sqrt(32) since q is
                layernormed and rstd cancels the key norm, so exp <= ~300),
                attn transpose on the sync ring, denominator reciprocal."""
                attn = quadmid.tile([128, 512], dt.bfloat16, tag="attn")
                denom = quadmid.tile([128, 1], dt.float32, tag="denom")
                nc.scalar.activation(
                    out=attn, in_=st["lnv"],
                    func=mybir.ActivationFunctionType.Exp,
                    accum_out=denom,
                )
                attnt = quadmid.tile([128, 4, 128], dt.bfloat16, tag="attnt")
                nc.sync.dma_start_transpose(out=attnt, in_=attn)
                rden = quadmid.tile([128, 1], dt.float32, tag="rden")
                nc.vector.reciprocal(out=rden, in_=denom)
                st["attnt"] = attnt
                st["rden"] = rden

            def emit_hv(g16):
                ctxt_sb = state.pop(("ctxt", g16))
                yt_ps = psA.tile([128, 512], dt.float32, tag="acc")
                for hp in range(H):
                    m, gpos = divmod(hp, 4)
                    for c in range(2):
                        nc.tensor.matmul(
                            out=yt_ps[32 * gpos:32 * gpos + 32,
                                      16 * m:16 * m + 16],
                            lhsT=wvt_sb[:, c, hp],
                            rhs=ctxt_sb[:, c::2, hp::32],
                            start=(c == 0), stop=(c == 1),
                            tile_position=(0, 32 * gpos),
                        )
                for m in range(2):
                    nc.vector.tensor_copy(
                        yt_sb[:, m, g16], yt_ps[:, 16 * m:16 * m + 16])

            def emit_ctx(quad, st):
                """ctx matmuls at the HEAD of the PE stream (inputs one full
                iteration old); hv for the previous finished group."""
                qq = quad % 4
                g16 = quad // 4
                if qq == 0:
                    state[("ctx4", g16)] = grp.tile(
                        [128, 4, 256], dt.bfloat16, tag="ctx4",
                        name="ctx4_sb")
                attnt = st["attnt"]
                ctx_ps = psA.tile([128, 512], dt.float32, tag="acc")
                for kk in range(4):
                    for b in range(4):
                        nc.tensor.matmul(
                            out=ctx_ps[32 * b:32 * b + 32, 0:256],
                            lhsT=attnt[:, kk, 32 * b:32 * b + 32],
                            rhs=st["xvt"][:, st["boff"] + b, kk],
                            start=(kk == 0), stop=(kk == 3),
                            tile_position=(0, 32 * b),
                        )
                st["ctx_ps"] = ctx_ps
                if qq == 0 and g16 > 0:
                    emit_hv(g16 - 1)

            def emit_ctx_post(quad, st):
                """ctx scale on DVE (ctx matmuls long done) + the batched
                group transpose on the scalar ring every 4th quad."""
                qq = quad % 4
                g16 = quad // 4
                ctx4 = state[("ctx4", g16)]
                nc.vector.tensor_scalar_mul(ctx4[:, qq], st["ctx_ps"][:, 0:256],
                                            st["rden"])
                if qq == 3:
                    ctxt_sb = grp.tile([128, 8, 128], dt.bfloat16,
                                       tag="ctxt", name="ctxt_sb")
                    nc.scalar.dma_start_transpose(
                        out=ctxt_sb,
                        in_=ctx4.rearrange("p q d -> p (q d)"))
                    state[("ctxt", g16)] = ctxt_sb
                    state.pop(("ctx4", g16))

            sts = {}
            issue_load(0)
            issue_load(1)
            for i in range(QUADS + 5):
                if i % 2 == 0:
                    issue_load(i // 2 + 2)
                if 3 <= i <= QUADS + 2:
                    emit_tail(sts[i - 3])
                if 2 <= i <= QUADS + 1:
                    emit_msq_ln(sts[i - 2])
                if 4 <= i <= QUADS + 3:
                    emit_ctx(i - 4, sts[i - 4])
                if i < QUADS:
                    sts[i] = emit_kproj(i)
                if 1 <= i <= QUADS:
                    emit_raw(sts[i - 1])
                if 2 <= i <= QUADS + 1:
                    emit_logits(sts[i - 2])
                if 4 <= i <= QUADS + 3:
                    emit_ctx_post(i - 4, sts[i - 4])
                    sts.pop(i - 4)
                    oct_tiles.pop((i - 5) // 2, None)
            emit_hv(GROUPS - 1)

            # ================= epilogue (whole core) =================
            # y natural: [batch, chunk, feat%128]
            y_nat = singles.tile([128, 2, 128], dt.bfloat16)
            nc.sync.dma_start_transpose(
                out=y_nat, in_=yt_sb.rearrange("p c g b -> p (c g b)")
            )
            stats = singles.tile([128, 6], dt.float32)
            nc.vector.bn_stats(out=stats, in_=y_nat.rearrange("p c d -> p (c d)"))
            mv = singles.tile([128, 2], dt.float32)
            nc.vector.bn_aggr(out=mv, in_=stats)
            std = singles.tile([128, 1], dt.float32)
            nc.scalar.activation(
                out=std, in_=mv[:, 1:2],
                func=mybir.ActivationFunctionType.Sqrt, bias=eps_sb,
            )
            rstd_y = singles.tile([128, 1], dt.float32)
            nc.vector.reciprocal(out=rstd_y, in_=std)
            y_ln = singles.tile([128, 2, 128], dt.bfloat16)
            nc.vector.tensor_scalar(
                out=y_ln, in0=y_nat,
                scalar1=mv[:, 0:1], scalar2=rstd_y,
                op0=mybir.AluOpType.subtract, op1=mybir.AluOpType.mult,
            )
            y_lnt = singles.tile([128, 2, 128], dt.bfloat16)
            nc.sync.dma_start_transpose(
                out=y_lnt, in_=y_ln.rearrange("p c d -> p (c d)")
            )
            outt_sb = singles.tile([128, 2, 128], dt.float32)
            for m in range(2):
                out_ps = psR.tile([128, 512], dt.float32, tag="raw")
                for c in range(2):
                    nc.tensor.matmul(
                        out=out_ps[:, 0:128],
                        lhsT=wpt_sb[:, c, m],
                        rhs=y_lnt[:, c],
                        start=(c == 0), stop=(c == 1),
                    )
                nc.vector.tensor_scalar_add(outt_sb[:, m], out_ps[:, 0:128], bias_sb[:, m:m + 1])
            # transpose back to natural via PE (f32)
            onat_sb = singles.tile([128, 2, 128], dt.float32)
            for m in range(2):
                tp_ps = psM.tile([128, 512], dt.float32, tag="msq")
                nc.tensor.transpose(out=tp_ps[:, 0:128], in_=outt_sb[:, m], identity=ident_sb)
                nc.scalar.copy(out=onat_sb[:, m], in_=tp_ps[:, 0:128])
            nc.sync.dma_start(
                out=out_d.rearrange("b (c d) -> b c d", c=2), in_=onat_sb
            )

    nc.compile()
    return nc


def _host_precompute(inputs):
    f32 = np.float32
    x_q = np.asarray(inputs["x_q"], f32)
    Wq = np.asarray(inputs["Wq"], f32)
    Wk = np.asarray(inputs["Wk"], f32)
    Wv = np.asarray(inputs["Wv"], f32)
    Wp = np.asarray(inputs["Wp"], f32)
    qn_w = np.asarray(inputs["qn_w"], f32)
    qn_b = np.asarray(inputs["qn_b"], f32)
    kn_w = np.asarray(inputs["kn_w"], f32)
    kn_b = np.asarray(inputs["kn_b"], f32)
    n_w = np.asarray(inputs["n_w"], f32)
    n_b = np.asarray(inputs["n_b"], f32)

    scale = HD ** -0.5
    q = (x_q @ Wq.T).reshape(B, H, HD)
    qmu = q.mean(-1, keepdims=True)
    qvar = q.var(-1, keepdims=True)
    qhat = (q - qmu) / np.sqrt(qvar + EPS) * qn_w + qn_b
    qhat = qhat * scale
    qw = qhat * kn_w
    s_qw = qw.sum(-1)
    Wk_h = Wk.reshape(H, HD, DIM)
    U = np.einsum("bhd,hdc->bhc", qw, Wk_h)
    M = Wk_h.mean(1)                       # (H, D)
    Up = U - s_qw[..., None] * M           # (B, H, D)
    # centered K-projection weights: k_c = x @ Wk_c.T has zero head-mean,
    # so var = mean(k_c^2) and raw' = U'.x directly (no mu stream on device)
    Wk_c = (Wk_h - M[:, None, :]).reshape(DIM, DIM)

    bf16 = ml_dtypes.bfloat16
    # up: [p, c, b_local, 32] per core (cols 8..32 zero-padded)
    Up_r = Up.reshape(B, H, 2, 128)        # (B,H,c,p)
    up_all = np.zeros((128, 2, B, 32), np.float32)
    up_all[:, :, :, :H] = Up_r.transpose(3, 2, 0, 1)
    up_all = up_all.astype(bf16)
    # wkt: [p=c_in%128, c, m, j]  lhsT[cfeat, outfeat] = Wk_c[128m+j, 128c+p]
    wkt = np.zeros((128, 2, 2, 128), f32)
    for c in range(2):
        for m in range(2):
            wkt[:, c, m, :] = Wk_c[128 * m:128 * m + 128, 128 * c:128 * c + 128].T
    wkt = wkt.astype(bf16)
    # on: [p, c, h] = 1/32 if head(128c+p)==h, padded to 32 cols
    on = np.zeros((128, 2, 32), f32)
    for h in range(H):
        c, lo = divmod(32 * h, 128)
        on[lo:lo + 32, c, h] = 1.0 / HD
    on = on.astype(bf16)
    # wvt: [p, c, h, d] = Wv[32h+d, 128c+p]
    wvt = np.ascontiguousarray(
        Wv.reshape(H, HD, 2, 128).transpose(3, 2, 0, 1)
    ).astype(bf16)
    # wpt: [p, c, m, j] = Wp[128m+j, 128c+p] * n_w[128c+p]
    Wpw = Wp * n_w[None, :]
    wpt = np.zeros((128, 2, 2, 128), f32)
    for c in range(2):
        for m in range(2):
            wpt[:, c, m, :] = Wpw[128 * m:128 * m + 128, 128 * c:128 * c + 128].T
    wpt = wpt.astype(bf16)
    bias_out = (Wp @ n_b).astype(f32)       # (256,)
    bias = np.ascontiguousarray(bias_out.reshape(2, 128).T)  # [p, m]
    ident = np.eye(128, dtype=f32)
    return up_all, wkt, on, wvt, wpt, bias, ident


def _host_pack_acts(inputs):
    """Cast x_k/x_v to bf16 and permute into one packed per-octet array in
    the exact SBUF tile layout, so each octet is a single contiguous DMA."""
    bf16 = ml_dtypes.bfloat16
    x_k = np.asarray(inputs["x_k"], np.float32).astype(bf16)
    x_v = np.asarray(inputs["x_v"], np.float32).astype(bf16)
    xall = np.empty((NCORES, OCTS, 128, 16384), bf16)
    # xkt[core, o, dp, b, kp, c, p'] = x_k[128*core+8o+b, 4p'+kp, 128c+dp]
    xk_r = x_k.reshape(NCORES, OCTS, 8, 128, 4, 2, 128)
    xall[..., :8192] = xk_r.transpose(0, 1, 6, 2, 4, 5, 3).reshape(
        NCORES, OCTS, 128, 8192)
    # xvt[core, o, p', b, kp, d] = x_v[128*core+8o+b, 4p'+kp, d]
    xv_r = x_v.reshape(NCORES, OCTS, 8, 128, 4, 256)
    xall[..., 8192:] = xv_r.transpose(0, 1, 3, 2, 4, 5).reshape(
        NCORES, OCTS, 128, 8192)
    return xall


def _make_in_maps(inputs):
    up_all, wkt, on, wvt, wpt, bias, ident = _host_precompute(inputs)
    xall = _host_pack_acts(inputs)
    in_maps = []
    for core in range(NCORES):
        sl = slice(core * BL, (core + 1) * BL)
        in_maps.append({
            "xall": xall[core],
            "up": np.ascontiguousarray(up_all[:, :, sl, :]),
            "wkt": wkt, "on": on,
            "wvt": wvt, "wpt": wpt, "bias": bias, "ident": ident,
        })
    return in_maps


def kernel(**inputs) -> np.ndarray:
    sys.path.insert(0, "/opt/trn_rl_repo")
    import concourse.bass as bass_mod
    import concourse.tile as tile_mod
    from concourse import mybir
    from concourse.bass_utils import run_bass_kernel_spmd

    if "nc" not in _CACHE:
        _CACHE["nc"] = _build(bass_mod, tile_mod, mybir)
    nc = _CACHE["nc"]

    in_maps = _make_in_maps(inputs)
    res = run_bass_kernel_spmd(nc, in_maps, core_ids=list(range(NCORES)))
    out = np.concatenate([res.results[i]["out"] for i in range(NCORES)], axis=0)
    return out.astype(np.float32)


if __name__ == "__main__":
    import reference
    inputs = reference.setup_inputs()
    inputs = {k: np.asarray(v) for k, v in inputs.items()}
    expected = np.asarray(reference.reference(**inputs))
    actual = kernel(**inputs)
    rel = np.linalg.norm(actual - expected) / np.linalg.norm(expected)
    print("Relative error:", rel)


# revision 22
# speedup vs baseline: 1.2240x; 1.2240x over previous
"""Trainium2 Bass kernel for nn_Attention_32246614458829.

Single-query attention with per-head QK LayerNorm, B=1024, K=512, D=256,
H=8 heads of 32 dims. Pure data parallel over 8 NeuronCores (128 batches
per core); weights replicated.

Math reformulation (validated against the reference in numpy):
  Scores need k = LN_head(x_k @ Wk.T).  Under the softmax, per-(b,h)
  constants drop, so with
      qhat = LN(q)*scale,  qw = qhat*kn_w,  s_qw = sum(qw),
      U'[b,h,:] = sum_d qw[b,h,d]*Wk[32h+d,:] - s_qw[b,h]*M[h,:],
      M[h,:]    = mean_d Wk[32h+d,:],
  the pre-softmax score is
      rstd * raw'   with  raw' = U'.x,
      var = meansq = mean_d (x @ Wk_c.T)^2   (Wk_c head-mean-centered),
      rstd = exp(-0.5*ln(var+eps)).
  The V projection is deferred until after the attention contraction:
      ctx[b,h,:] = sum_i attn[b,h,i]*x_v[b,i,:]   (256-dim)
      y[b,32h+d] = ctx[b,h,:] . Wv[32h+d,:]
      out = LN(y)*n_w @ Wp.T + n_b @ Wp.T  (n_w folded into Wp on host)

v2 data path: x_k / x_v are cast to bf16 AND laid out on the host in the
exact feature-major SBUF tile layouts the matmuls consume.  This halves
HBM traffic (f32 -> bf16) and removes every on-chip DMA transpose; each
octet of 8 batches is two plain 2 MiB HWDGE streams with 16 KiB
contiguous per-partition lines.  Matmul issue order is wave-packed
across the 4 PE column groups (scores and ctx), squares are split
scalar/DVE, and the softmax tail of quad q-1 is emitted at the head of
iteration q so the PE never waits on the activation chain.
"""

import os
import sys
import math

import numpy as np
import ml_dtypes

sys.path.insert(0, "/opt/trn_rl_repo")

DIM, H, HD = 256, 8, 32
EPS = 1e-5
B, K = 1024, 512
NCORES = 8
BL = B // NCORES          # 128 batches per core
QUADS = BL // 4           # 32 quads of 4 batches
GROUPS = BL // 16         # 8 groups of 16 batches (for the hv stage)
OCTS = BL // 8            # 16 octets of 8 batches

_CACHE = {}


def _build(nc_mod, tile_mod, mybir):
    bass = nc_mod
    dt = mybir.dt
    nc = __import__("concourse.bacc", fromlist=["Bacc"]).Bacc(
        "TRN2",
        target_bir_lowering=False,
        debug=False,
        enable_asserts=False,
        num_devices=NCORES,
    )

    # ---- DRAM parameters (per-core shards / replicated consts) ----
    # host-pretransposed activations (bf16), packed per octet so each octet
    # is ONE contiguous 4 MiB DMA (32 KiB per partition line):
    #   xall[o, p, 0:8192]     = xkt[o, dp, b, kp, c, p']
    #                            (= x_k[8o+b, 4p'+kp, 128c+dp], p = dp)
    #   xall[o, p, 8192:16384] = xvt[o, p', b, kp, d]
    #                            (= x_v[8o+b, 4p'+kp, d], p = p')
    xall_d = nc.dram_tensor("xall", [OCTS, 128, 16384], dt.bfloat16,
                            kind="ExternalInput").ap()
    up_d = nc.dram_tensor("up", [128, 2, BL, 32], dt.bfloat16, kind="ExternalInput").ap()
    wkt_d = nc.dram_tensor("wkt", [128, 2, 2, 128], dt.bfloat16, kind="ExternalInput").ap()
    on_d = nc.dram_tensor("on", [128, 2, 32], dt.bfloat16, kind="ExternalInput").ap()
    wvt_d = nc.dram_tensor("wvt", [128, 2, 8, 32], dt.bfloat16, kind="ExternalInput").ap()
    wpt_d = nc.dram_tensor("wpt", [128, 2, 2, 128], dt.bfloat16, kind="ExternalInput").ap()
    bias_d = nc.dram_tensor("bias", [128, 2], dt.float32, kind="ExternalInput").ap()
    ident_d = nc.dram_tensor("ident", [128, 128], dt.float32, kind="ExternalInput").ap()
    identb_d = nc.dram_tensor("identb", [128, 128], dt.bfloat16, kind="ExternalInput").ap()
    out_d = nc.dram_tensor("out", [BL, DIM], dt.float32, kind="ExternalOutput").ap()

    with tile_mod.TileContext(nc) as tc:
        with tc.tile_pool(name="singles", bufs=1) as singles, \
             tc.tile_pool(name="octk", bufs=3) as octk, \
             tc.tile_pool(name="octv", bufs=6) as octv, \
             tc.tile_pool(name="upio", bufs=4) as upio, \
             tc.tile_pool(name="quadmid", bufs=3) as quadmid, \
             tc.tile_pool(name="grp", bufs=2) as grp, \
             tc.tile_pool(name="psA", bufs=1, space="PSUM") as psA, \
             tc.tile_pool(name="psK", bufs=3, space="PSUM") as psK, \
             tc.tile_pool(name="psR", bufs=2, space="PSUM") as psR, \
             tc.tile_pool(name="psM", bufs=1, space="PSUM") as psM, \
             tc.tile_pool(name="psT", bufs=1, space="PSUM") as psT:

            # Pin the activation table set to natural_log_exp_and_others
            # (global set id 6): it contains square, ln AND exp, so the
            # whole main loop runs with zero ACT_TABLE_LOAD swaps.
            nc.scalar.add_instruction(mybir.InstLoadActFuncSet(
                name=nc.scalar.bass.get_next_instruction_name(),
                ins=[], outs=[], act_func_set_id=6,
            ))

            # ---- load constants once ----
            wkt_sb = singles.tile([128, 2, 2, 128], dt.bfloat16)
            nc.sync.dma_start(out=wkt_sb, in_=wkt_d)
            on_sb = singles.tile([128, 2, 32], dt.bfloat16)
            nc.sync.dma_start(out=on_sb, in_=on_d)
            wvt_sb = singles.tile([128, 2, 8, 32], dt.bfloat16)
            nc.sync.dma_start(out=wvt_sb, in_=wvt_d)
            wpt_sb = singles.tile([128, 2, 2, 128], dt.bfloat16)
            nc.sync.dma_start(out=wpt_sb, in_=wpt_d)
            bias_sb = singles.tile([128, 2], dt.float32)
            nc.sync.dma_start(out=bias_sb, in_=bias_d)
            ident_sb = singles.tile([128, 128], dt.float32)
            nc.sync.dma_start(out=ident_sb, in_=ident_d)
            identb_sb = singles.tile([128, 128], dt.bfloat16)
            nc.sync.dma_start(out=identb_sb, in_=identb_d)

            # y.T accumulator for the whole core: [feat%128, chunk, group, 16]
            yt_sb = singles.tile([128, 2, GROUPS, 16], dt.bfloat16)
            eps_sb = singles.tile([128, 1], dt.float32)
            nc.vector.memset(eps_sb, EPS)

            oct_tiles = {}
            state = {}

            def issue_load(o):
                """issue octet o's loads 2 octets ahead of first use, on
                three parallel DMA paths (xkt SWDGE, xvt sync HWDGE, up
                scalar HWDGE).  No xbar DMA transposes exist anywhere in the
                main loop (the Tile scheduler serializes them behind every
                outstanding DMA), so the load streams are unconstrained."""
                if o >= OCTS:
                    return
                xkt8 = octk.tile([128, 8, 4, 2, 128], dt.bfloat16,
                                 tag="xkt", name="xkt8")
                nc.gpsimd.dma_start(
                    out=xkt8,
                    in_=xall_d[o][:, 0:8192].rearrange(
                        "p (b kp c q) -> p b kp c q", b=8, kp=4, c=2, q=128))
                xvt8 = octv.tile([128, 8, 4, 256], dt.bfloat16,
                                 tag="xvt", name="xvt8")
                nc.sync.dma_start(
                    out=xvt8,
                    in_=xall_d[o][:, 8192:16384].rearrange(
                        "p (b kp d) -> p b kp d", b=8, kp=4, d=256))
                up8 = upio.tile([128, 2, 8, 32], dt.bfloat16,
                                tag="up8", name="up8")
                nc.scalar.dma_start(out=up8, in_=up_d[:, :, 8 * o:8 * o + 8])
                oct_tiles[o] = (xkt8, xvt8, up8)

            # ---- per-quad pipeline stages (quad q) ----
            # S0 (iter q):   kproj + squares (DVE cast, gp/DVE mul, scalar sq)
            # S1 (iter q+1): raw score waves [PE]
            # S2 (iter q+2): msq waves [PE], ln/rstd [scalar], logits [DVE]
            # S3 (iter q+3): exp [scalar], recip [DVE]; attn.T via PE at the
            #                TAIL of the same iteration's PE stream + DVE copy
            # S4 (iter q+4): ctx [PE head], ctx scale [DVE], ctx.T [PE tail]
            # Every engine reaches each op with its inputs already ready.

            def emit_kproj(quad):
                o, half = divmod(quad, 2)
                xkt, xvt, up8 = oct_tiles[o]
                boff = half * 4
                ksqt = quadmid.tile([128, 2, 4, 4, 128], dt.bfloat16, tag="ksqt")
                for b in range(4):
                    for m in range(2):
                        kt_ps = psK.tile([128, 512], dt.float32, tag="kt")
                        for c in range(2):
                            nc.tensor.matmul(
                                out=kt_ps,
                                lhsT=wkt_sb[:, c, m],
                                rhs=xkt[:, boff + b, :, c, :],
                                start=(c == 0), stop=(c == 1),
                            )
                        idx = b * 2 + m
                        if idx < 5:
                            ktc = quadmid.tile([128, 512], dt.bfloat16,
                                               tag="ktc")
                            nc.vector.tensor_copy(ktc, kt_ps)
                            if idx < 4:
                                nc.gpsimd.tensor_mul(ksqt[:, m, b], ktc, ktc)
                            else:
                                nc.vector.tensor_mul(ksqt[:, m, b], ktc, ktc)
                        else:
                            nc.scalar.activation(
                                out=ksqt[:, m, b], in_=kt_ps,
                                func=mybir.ActivationFunctionType.Square,
                            )
                return {"ksqt": ksqt, "xkt": xkt, "xvt": xvt, "up8": up8,
                        "boff": boff, "quad": quad}

            def emit_raw(st):
                raw_ps = psR.tile([128, 512], dt.float32, tag="raw")
                for c in range(2):
                    for b in range(4):
                        nc.tensor.matmul(
                            out=raw_ps[32 * b:32 * b + 32],
                            lhsT=st["up8"][:, c, st["boff"] + b],
                            rhs=st["xkt"][:, st["boff"] + b, :, c, :],
                            start=(c == 0), stop=(c == 1),
                            tile_position=(0, 32 * b),
                        )
                st["raw"] = raw_ps

            def emit_msq_ln(st):
                msq_ps = psM.tile([128, 512], dt.float32, tag="msq")
                for c in range(2):
                    for b in range(4):
                        nc.tensor.matmul(
                            out=msq_ps[32 * b:32 * b + 32],
                            lhsT=on_sb[:, c],
                            rhs=st["ksqt"][:, c, b],
                            start=(c == 0), stop=(c == 1),
                            tile_position=(0, 32 * b),
                        )
                lnv = quadmid.tile([128, 512], dt.float32, tag="lnv")
                nc.scalar.activation(
                    out=lnv, in_=msq_ps,
                    func=mybir.ActivationFunctionType.Ln, bias=eps_sb,
                )
                nc.scalar.activation(
                    out=lnv, in_=lnv,
                    func=mybir.ActivationFunctionType.Exp, scale=-0.5,
                )
                st["lnv"] = lnv

            def emit_logits(st):
                nc.vector.tensor_mul(st["lnv"], st["raw"], st["lnv"])

            def emit_tail(st):
                """exp (no max subtraction: |logit| <= sqrt(32) since q is
                layernormed and rstd cancels the key norm, so exp <= ~300)
                and the denominator reciprocal."""
                attn = quadmid.tile([128, 512], dt.bfloat16, tag="attn")
                denom = quadmid.tile([128, 1], dt.float32, tag="denom")
                nc.scalar.activation(
                    out=attn, in_=st["lnv"],
                    func=mybir.ActivationFunctionType.Exp,
                    accum_out=denom,
                )
                rden = quadmid.tile([128, 1], dt.float32, tag="rden")
                nc.vector.reciprocal(out=rden, in_=denom)
                st["attn"] = attn
                st["rden"] = rden

            def emit_atp(st, tp_ps):
                """attn.T via PE transposes at the tail of the PE stream
                (attn was produced by the scalar exp at this iteration's
                head, so the PE never waits), then the DVE copy to SBUF."""
                for j in range(4):
                    nc.tensor.transpose(
                        out=tp_ps[:, j],
                        in_=st["attn"][:, 128 * j:128 * j + 128],
                        identity=identb_sb)
                attnt = quadmid.tile([128, 4, 128], dt.bfloat16, tag="attnt")
                nc.vector.tensor_copy(attnt, tp_ps[:, 0:4])
                st["attnt"] = attnt

            def emit_hv(g16):
                ctxt_sb = state.pop(("ctxt", g16))
                yt_ps = psA.tile([128, 512], dt.float32, tag="acc")
                for hp in range(H):
                    m, gpos = divmod(hp, 4)
                    for c in range(2):
                        nc.tensor.matmul(
                            out=yt_ps[32 * gpos:32 * gpos + 32,
                                      16 * m:16 * m + 16],
                            lhsT=wvt_sb[:, c, hp],
                            rhs=ctxt_sb[:, :, c, hp::32],
                            start=(c == 0), stop=(c == 1),
                            tile_position=(0, 32 * gpos),
                        )
                for m in range(2):
                    nc.vector.tensor_copy(
                        yt_sb[:, m, g16], yt_ps[:, 16 * m:16 * m + 16])

            def emit_ctx(quad, st):
                """ctx matmuls at the HEAD of the PE stream (inputs one full
                iteration old); hv for the previous finished group."""
                qq = quad % 4
                g16 = quad // 4
                if qq == 0:
                    state[("ctxt", g16)] = grp.tile(
                        [128, 4, 2, 128], dt.bfloat16, tag="ctxt",
                        name="ctxt_sb")
                attnt = st["attnt"]
                ctx_ps = psA.tile([128, 512], dt.float32, tag="acc")
                for kk in range(4):
                    for b in range(4):
                        nc.tensor.matmul(
                            out=ctx_ps[32 * b:32 * b + 32, 0:256],
                            lhsT=attnt[:, kk, 32 * b:32 * b + 32],
                            rhs=st["xvt"][:, st["boff"] + b, kk],
                            start=(kk == 0), stop=(kk == 3),
                            tile_position=(0, 32 * b),
                        )
                st["ctx_ps"] = ctx_ps
                if qq == 0 and g16 > 0:
                    emit_hv(g16 - 1)

            def emit_ctx_post(quad, st, tp_ps):
                """ctx scale on DVE, then ctx.T via PE transposes at the PE
                stream tail and the DVE copy into the group ctxt tile."""
                qq = quad % 4
                g16 = quad // 4
                ctx_sb = quadmid.tile([128, 256], dt.bfloat16, tag="ctxsb")
                nc.vector.tensor_scalar_mul(ctx_sb, st["ctx_ps"][:, 0:256],
                                            st["rden"])
                for j in range(2):
                    nc.tensor.transpose(
                        out=tp_ps[:, 4 + j],
                        in_=ctx_sb[:, 128 * j:128 * j + 128],
                        identity=identb_sb)
                ctxt_sb = state[("ctxt", g16)]
                nc.vector.tensor_copy(ctxt_sb[:, qq], tp_ps[:, 4:6])

            sts = {}
            issue_load(0)
            issue_load(1)
            for i in range(QUADS + 5):
                if i % 2 == 0:
                    issue_load(i // 2 + 2)
                if 3 <= i <= QUADS + 2:
                    emit_tail(sts[i - 3])
                if 2 <= i <= QUADS + 1:
                    emit_msq_ln(sts[i - 2])
                if 4 <= i <= QUADS + 3:
                    emit_ctx(i - 4, sts[i - 4])
                if i < QUADS:
                    sts[i] = emit_kproj(i)
                if 1 <= i <= QUADS:
                    emit_raw(sts[i - 1])
                if 2 <= i <= QUADS + 1:
                    emit_logits(sts[i - 2])
                if 3 <= i <= QUADS + 3:
                    tp_ps = psT.tile([128, 6, 128], dt.bfloat16, tag="tp",
                                     name="tp_ps")
                    if i <= QUADS + 2:
                        emit_atp(sts[i - 3], tp_ps)
                    if i >= 4:
                        emit_ctx_post(i - 4, sts[i - 4], tp_ps)
                        sts.pop(i - 4)
                        oct_tiles.pop((i - 5) // 2, None)
            emit_hv(GROUPS - 1)

            # ================= epilogue (whole core) =================
            # y natural: [batch, chunk, feat%128]
            y_nat = singles.tile([128, 2, 128], dt.bfloat16)
            nc.sync.dma_start_transpose(
                out=y_nat, in_=yt_sb.rearrange("p c g b -> p (c g b)")
            )
            stats = singles.tile([128, 6], dt.float32)
            nc.vector.bn_stats(out=stats, in_=y_nat.rearrange("p c d -> p (c d)"))
            mv = singles.tile([128, 2], dt.float32)
            nc.vector.bn_aggr(out=mv, in_=stats)
            std = singles.tile([128, 1], dt.float32)
            nc.scalar.activation(
                out=std, in_=mv[:, 1:2],
                func=mybir.ActivationFunctionType.Sqrt, bias=eps_sb,
            )
            rstd_y = singles.tile([128, 1], dt.float32)
            nc.vector.reciprocal(out=rstd_y, in_=std)
            y_ln = singles.tile([128, 2, 128], dt.bfloat16)
            nc.vector.tensor_scalar(
                out=y_ln, in0=y_nat,
                scalar1=mv[:, 0:1], scalar2=rstd_y,
                op0=mybir.AluOpType.subtract, op1=mybir.AluOpType.mult,
            )
            y_lnt = singles.tile([128, 2, 128], dt.bfloat16)
            nc.sync.dma_start_transpose(
                out=y_lnt, in_=y_ln.rearrange("p c d -> p (c d)")
            )
            outt_sb = singles.tile([128, 2, 128], dt.float32)
            for m in range(2):
                out_ps = psR.tile([128, 512], dt.float32, tag="raw")
                for c in range(2):
                    nc.tensor.matmul(
                        out=out_ps[:, 0:128],
                        lhsT=wpt_sb[:, c, m],
                        rhs=y_lnt[:, c],
                        start=(c == 0), stop=(c == 1),
                    )
                nc.vector.tensor_scalar_add(outt_sb[:, m], out_ps[:, 0:128], bias_sb[:, m:m + 1])
            # transpose back to natural via PE (f32)
            onat_sb = singles.tile([128, 2, 128], dt.float32)
            for m in range(2):
                tp_ps = psM.tile([128, 512], dt.float32, tag="msq")
                nc.tensor.transpose(out=tp_ps[:, 0:128], in_=outt_sb[:, m], identity=ident_sb)
                nc.scalar.copy(out=onat_sb[:, m], in_=tp_ps[:, 0:128])
            nc.sync.dma_start(
                out=out_d.rearrange("b (c d) -> b c d", c=2), in_=onat_sb
            )

    nc.compile()
    return nc


def _host_precompute(inputs):
    f32 = np.float32
    x_q = np.asarray(inputs["x_q"], f32)
    Wq = np.asarray(inputs["Wq"], f32)
    Wk = np.asarray(inputs["Wk"], f32)
    Wv = np.asarray(inputs["Wv"], f32)
    Wp = np.asarray(inputs["Wp"], f32)
    qn_w = np.asarray(inputs["qn_w"], f32)
    qn_b = np.asarray(inputs["qn_b"], f32)
    kn_w = np.asarray(inputs["kn_w"], f32)
    kn_b = np.asarray(inputs["kn_b"], f32)
    n_w = np.asarray(inputs["n_w"], f32)
    n_b = np.asarray(inputs["n_b"], f32)

    scale = HD ** -0.5
    q = (x_q @ Wq.T).reshape(B, H, HD)
    qmu = q.mean(-1, keepdims=True)
    qvar = q.var(-1, keepdims=True)
    qhat = (q - qmu) / np.sqrt(qvar + EPS) * qn_w + qn_b
    qhat = qhat * scale
    qw = qhat * kn_w
    s_qw = qw.sum(-1)
    Wk_h = Wk.reshape(H, HD, DIM)
    U = np.einsum("bhd,hdc->bhc", qw, Wk_h)
    M = Wk_h.mean(1)                       # (H, D)
    Up = U - s_qw[..., None] * M           # (B, H, D)
    # centered K-projection weights: k_c = x @ Wk_c.T has zero head-mean,
    # so var = mean(k_c^2) and raw' = U'.x directly (no mu stream on device)
    Wk_c = (Wk_h - M[:, None, :]).reshape(DIM, DIM)

    bf16 = ml_dtypes.bfloat16
    # up: [p, c, b_local, 32] per core (cols 8..32 zero-padded)
    Up_r = Up.reshape(B, H, 2, 128)        # (B,H,c,p)
    up_all = np.zeros((128, 2, B, 32), np.float32)
    up_all[:, :, :, :H] = Up_r.transpose(3, 2, 0, 1)
    up_all = up_all.astype(bf16)
    # wkt: [p=c_in%128, c, m, j]  lhsT[cfeat, outfeat] = Wk_c[128m+j, 128c+p]
    wkt = np.zeros((128, 2, 2, 128), f32)
    for c in range(2):
        for m in range(2):
            wkt[:, c, m, :] = Wk_c[128 * m:128 * m + 128, 128 * c:128 * c + 128].T
    wkt = wkt.astype(bf16)
    # on: [p, c, h] = 1/32 if head(128c+p)==h, padded to 32 cols
    on = np.zeros((128, 2, 32), f32)
    for h in range(H):
        c, lo = divmod(32 * h, 128)
        on[lo:lo + 32, c, h] = 1.0 / HD
    on = on.astype(bf16)
    # wvt: [p, c, h, d] = Wv[32h+d, 128c+p]
    wvt = np.ascontiguousarray(
        Wv.reshape(H, HD, 2, 128).transpose(3, 2, 0, 1)
    ).astype(bf16)
    # wpt: [p, c, m, j] = Wp[128m+j, 128c+p] * n_w[128c+p]
    Wpw = Wp * n_w[None, :]
    wpt = np.zeros((128, 2, 2, 128), f32)
    for c in range(2):
        for m in range(2):
            wpt[:, c, m, :] = Wpw[128 * m:128 * m + 128, 128 * c:128 * c + 128].T
    wpt = wpt.astype(bf16)
    bias_out = (Wp @ n_b).astype(f32)       # (256,)
    bias = np.ascontiguousarray(bias_out.reshape(2, 128).T)  # [p, m]
    ident = np.eye(128, dtype=f32)
    identb = np.eye(128, dtype=f32).astype(bf16)
    return up_all, wkt, on, wvt, wpt, bias, ident, identb


def _host_pack_acts(inputs):
    """Cast x_k/x_v to bf16 and permute into one packed per-octet array in
    the exact SBUF tile layout, so each octet is a single contiguous DMA."""
    bf16 = ml_dtypes.bfloat16
    x_k = np.asarray(inputs["x_k"], np.float32).astype(bf16)
    x_v = np.asarray(inputs["x_v"], np.float32).astype(bf16)
    xall = np.empty((NCORES, OCTS, 128, 16384), bf16)
    # xkt[core, o, dp, b, kp, c, p'] = x_k[128*core+8o+b, 4p'+kp, 128c+dp]
    xk_r = x_k.reshape(NCORES, OCTS, 8, 128, 4, 2, 128)
    xall[..., :8192] = xk_r.transpose(0, 1, 6, 2, 4, 5, 3).reshape(
        NCORES, OCTS, 128, 8192)
    # xvt[core, o, p', b, kp, d] = x_v[128*core+8o+b, 4p'+kp, d]
    xv_r = x_v.reshape(NCORES, OCTS, 8, 128, 4, 256)
    xall[..., 8192:] = xv_r.transpose(0, 1, 3, 2, 4, 5).reshape(
        NCORES, OCTS, 128, 8192)
    return xall


def _make_in_maps(inputs):
    up_all, wkt, on, wvt, wpt, bias, ident, identb = _host_precompute(inputs)
    xall = _host_pack_acts(inputs)
    in_maps = []
    for core in range(NCORES):
        sl = slice(core * BL, (core + 1) * BL)
        in_maps.append({
            "xall": xall[core],
            "up": np.ascontiguousarray(up_all[:, :, sl, :]),
            "wkt": wkt, "on": on,
            "wvt": wvt, "wpt": wpt, "bias": bias, "ident": ident,
            "identb": identb,
        })
    return in_maps


def kernel(**inputs) -> np.ndarray:
    sys.path.insert(0, "/opt/trn_rl_repo")
    import concourse.bass as bass_mod
    import concourse.tile as tile_mod
    from concourse import mybir
    from concourse.bass_utils import run_bass_kernel_spmd

    if "nc" not in _CACHE:
        _CACHE["nc"] = _build(bass_mod, tile_mod, mybir)
    nc = _CACHE["nc"]

    in_maps = _make_in_maps(inputs)
    res = run_bass_kernel_spmd(nc, in_maps, core_ids=list(range(NCORES)))
    out = np.concatenate([res.results[i]["out"] for i in range(NCORES)], axis=0)
    return out.astype(np.float32)


if __name__ == "__main__":
    import reference
    inputs = reference.setup_inputs()
    inputs = {k: np.asarray(v) for k, v in inputs.items()}
    expected = np.asarray(reference.reference(**inputs))
    actual = kernel(**inputs)
    rel = np.linalg.norm(actual - expected) / np.linalg.norm(expected)
    print("Relative error:", rel)
